# revision 1
# baseline (speedup 1.0000x reference)
"""Trainium2 Bass kernel for a binarized-conv BasicBlock (2x BinConv3x3 + BN + residual + PReLU).

Strategy (8 NeuronCores, data-parallel over batch):
  - 64 images -> 8 per core; binarized conv weights / BN / PReLU params replicated.
  - Binarized values are exactly +/-1, so bf16 matmuls on the tensor engine are
    numerically exact (fp32 PSUM accumulation of small integers).
  - Conv3x3 as implicit GEMM: per output tile [128 Cout x 392 cols] accumulate
    2 Cin-blocks x 9 taps = 18 matmuls reading shifted windows of a zero-padded
    (30x30) binarized activation image.
  - BatchNorm uses full-batch statistics: per-channel sum/sumsq partials from
    bn_stats fused with PSUM evacuation, one tiny (2KB) AllReduce per BN.
  - PReLU runs on the scalar engine's Prelu activation with runtime alpha.
"""

import numpy as np
import ml_dtypes

import concourse.bacc as bacc
import concourse.mybir as mybir
import concourse.tile as tile
from concourse import bass_utils

N_CORES = 8
B_FULL, C, H, W = 64, 256, 28, 28
BL = B_FULL // N_CORES  # images per core
P = 128
NB = C // P             # channel blocks
HW = H * W              # 784
PADL = 30               # padded row length
PADQ = PADL * PADL      # 900 padded image
HALF = 14 * W           # 392 columns per psum tile (half an image)
NI_LOCAL = float(BL * HW)      # interior elems per core per channel
N_TOT = float(B_FULL * HW)     # full-batch elems per channel
SCALE = 0.1
BN_EPS = 1e-5

F32 = mybir.dt.float32
BF16 = mybir.dt.bfloat16
BF16_NP = np.dtype(ml_dtypes.bfloat16)

_CACHE: dict = {}


def _build():
    nc = bacc.Bacc("TRN2", target_bir_lowering=False, debug=False,
                   num_devices=N_CORES)
    F = mybir.ActivationFunctionType
    Op = mybir.AluOpType

    x_d = nc.dram_tensor("x", [BL, C, H, W], F32, kind="ExternalInput")
    w1_d = nc.dram_tensor("w1", [NB, P, 9, NB, P], BF16, kind="ExternalInput")
    w2_d = nc.dram_tensor("w2", [NB, P, 9, NB, P], BF16, kind="ExternalInput")
    # per-channel params packed [NB, P, 1]: col order s(=SCALE*mean|w|), gamma, beta
    s1_d = nc.dram_tensor("s1", [NB, P, 1], F32, kind="ExternalInput")
    g1_d = nc.dram_tensor("g1", [NB, P, 1], F32, kind="ExternalInput")
    be1_d = nc.dram_tensor("be1", [NB, P, 1], F32, kind="ExternalInput")
    s2_d = nc.dram_tensor("s2", [NB, P, 1], F32, kind="ExternalInput")
    g2_d = nc.dram_tensor("g2", [NB, P, 1], F32, kind="ExternalInput")
    be2_d = nc.dram_tensor("be2", [NB, P, 1], F32, kind="ExternalInput")
    a1_d = nc.dram_tensor("a1", [1], F32, kind="ExternalInput")
    a2_d = nc.dram_tensor("a2", [1], F32, kind="ExternalInput")
    o_d = nc.dram_tensor("o", [BL, C, H, W], F32, kind="ExternalOutput")

    with tile.TileContext(nc) as tc:
        with (
            tc.tile_pool(name="sbuf", bufs=1) as sbuf,
            tc.tile_pool(name="psum", bufs=6, space="PSUM") as psum_pool,
            tc.tile_pool(name="dram", bufs=1, space="DRAM") as dram,
        ):
            # ---- static parameters ----
            w1_sb = sbuf.tile([P, NB, 9, NB, P], BF16)
            w2_sb = sbuf.tile([P, NB, 9, NB, P], BF16)
            for k in range(NB):
                nc.sync.dma_start(w1_sb[:, k], w1_d[k])
                nc.sync.dma_start(w2_sb[:, k], w2_d[k])

            s1_sb = sbuf.tile([P, NB], F32)
            g1_sb = sbuf.tile([P, NB], F32)
            be1_sb = sbuf.tile([P, NB], F32)
            s2_sb = sbuf.tile([P, NB], F32)
            g2_sb = sbuf.tile([P, NB], F32)
            be2_sb = sbuf.tile([P, NB], F32)
            for sb, d in ((s1_sb, s1_d), (g1_sb, g1_d), (be1_sb, be1_d),
                          (s2_sb, s2_d), (g2_sb, g2_d), (be2_sb, be2_d)):
                for m in range(NB):
                    nc.sync.dma_start(sb[:, m:m + 1], d[m])
            a1_sb = sbuf.tile([P, 1], F32)
            a2_sb = sbuf.tile([P, 1], F32)
            nc.sync.dma_start(a1_sb[:], a1_d[None, :].partition_broadcast(P))
            nc.sync.dma_start(a2_sb[:], a2_d[None, :].partition_broadcast(P))

            # ---- activations ----
            xb_img = [sbuf.tile([P, NB, PADQ], BF16, name=f"xb{b}")
                      for b in range(BL)]
            x_img = [sbuf.tile([P, NB, HW], F32, name=f"xr{b}")
                     for b in range(BL)]
            y_img = [sbuf.tile([P, NB, HW], F32, name=f"yy{b}")
                     for b in range(BL)]
            xbv = [t.rearrange("p k (r c) -> p k r c", c=PADL) for t in xb_img]
            xv = [t.rearrange("p k (r c) -> p k r c", c=W) for t in x_img]
            yv = [t.rearrange("p k (r c) -> p k r c", c=W) for t in y_img]

            # load x, binarize into zeroed padded layout
            for b in range(BL):
                nc.gpsimd.memset(xb_img[b][:], 0.0)
                for k in range(NB):
                    for hh in range(2):
                        nc.sync.dma_start(
                            xv[b][:, k, hh * 14:(hh + 1) * 14, :],
                            x_d[b, k * P:(k + 1) * P, hh * 14:(hh + 1) * 14, :])
                    nc.scalar.sign(xbv[b][:, k, 1:29, 1:29], xv[b][:, k])

            def conv(w_sb, y_out, yo_view, st6):
                """bin-conv3x3; writes raw integer conv sums + per-tile stats."""
                for b in range(BL):
                    for m in range(NB):
                        for hh in range(2):
                            ps = psum_pool.tile([P, HALF], F32, name="ps",
                                                tag="ps")
                            n_mm = 0
                            for k in range(NB):
                                for dh in range(3):
                                    for dw in range(3):
                                        rhs = xbv[b][:, k,
                                                     hh * 14 + dh:hh * 14 + dh + 14,
                                                     dw:dw + 28]
                                        nc.tensor.matmul(
                                            ps[:], w_sb[:, k, dh * 3 + dw, m, :],
                                            rhs, start=(n_mm == 0),
                                            stop=(n_mm == 17))
                                        n_mm += 1
                            dst = y_out[b][:, m, hh * HALF:(hh + 1) * HALF]
                            nc.scalar.copy(dst, ps[:])
                            idx = (b * 2 + hh) * 6
                            nc.vector.bn_stats(st6[:, m, idx:idx + 6], dst)

            def stats_to_ab(st6, s_sb, g_sb, be_sb, tagn):
                """aggregate local stats, AllReduce, produce per-channel A,B."""
                st2 = sbuf.tile([P, NB, 2], F32, name=f"st2_{tagn}")
                for m in range(NB):
                    nc.vector.bn_aggr(st2[:, m], st6[:, m])
                cc_in = sbuf.tile([P, 4], F32, name=f"ccin_{tagn}")
                tmp = sbuf.tile([P, NB], F32, name=f"ctmp_{tagn}")
                # cols 0:2 = sum(y) per channel; 2:4 = sum(y^2)
                nc.vector.tensor_scalar(cc_in[:, 0:2], st2[:, :, 0], NI_LOCAL,
                                        None, Op.mult)
                nc.vector.tensor_tensor(tmp[:], st2[:, :, 0], st2[:, :, 0],
                                        Op.mult)
                nc.vector.tensor_tensor(tmp[:], st2[:, :, 1], tmp[:], Op.add)
                nc.vector.tensor_scalar(cc_in[:, 2:4], tmp[:], NI_LOCAL, None,
                                        Op.mult)

                cc_din = dram.tile([P, 4], F32, name=f"ccdin_{tagn}")
                cc_dout = dram.tile([P, 4], F32, name=f"ccdout_{tagn}",
                                    addr_space="Shared")
                nc.sync.dma_start(cc_din[:], cc_in[:])
                nc.gpsimd.collective_compute(
                    "AllReduce", Op.add,
                    replica_groups=[list(range(N_CORES))],
                    ins=[cc_din[:]], outs=[cc_dout[:]])
                cc_out = sbuf.tile([P, 4], F32, name=f"ccout_{tagn}")
                nc.sync.dma_start(cc_out[:], cc_dout[:])

                mg = sbuf.tile([P, NB], F32, name=f"mg_{tagn}")
                vg = sbuf.tile([P, NB], F32, name=f"vg_{tagn}")
                t0 = sbuf.tile([P, NB], F32, name=f"t0_{tagn}")
                d = sbuf.tile([P, NB], F32, name=f"d_{tagn}")
                r = sbuf.tile([P, NB], F32, name=f"r_{tagn}")
                av = sbuf.tile([P, NB], F32, name=f"av_{tagn}")
                bv = sbuf.tile([P, NB], F32, name=f"bv_{tagn}")
                nc.vector.tensor_scalar(mg[:], cc_out[:, 0:2], 1.0 / N_TOT,
                                        None, Op.mult)
                nc.vector.tensor_scalar(vg[:], cc_out[:, 2:4], 1.0 / N_TOT,
                                        None, Op.mult)
                nc.vector.tensor_tensor(t0[:], mg[:], mg[:], Op.mult)
                nc.vector.tensor_tensor(vg[:], vg[:], t0[:], Op.subtract)
                # d = s^2 * var_y + eps   (== var(out) + eps up to rounding)
                nc.vector.tensor_tensor(t0[:], s_sb[:], s_sb[:], Op.mult)
                nc.vector.tensor_tensor(d[:], t0[:], vg[:], Op.mult)
                nc.vector.tensor_scalar(d[:], d[:], BN_EPS, None, Op.add)
                # r = rsqrt(d): sqrt+divide, then one Newton step
                nc.scalar.sqrt(t0[:], d[:])
                nc.vector.reciprocal(r[:], t0[:])
                nc.vector.tensor_tensor(t0[:], r[:], r[:], Op.mult)
                nc.vector.tensor_tensor(t0[:], t0[:], d[:], Op.mult)
                nc.vector.tensor_scalar(t0[:], t0[:], -0.5, 1.5, Op.mult,
                                        Op.add)
                nc.vector.tensor_tensor(r[:], r[:], t0[:], Op.mult)
                # A = gamma * s * r ; B = beta - mean(out)*A/s... (B = beta - mg*A)
                nc.vector.tensor_tensor(av[:], g_sb[:], s_sb[:], Op.mult)
                nc.vector.tensor_tensor(av[:], av[:], r[:], Op.mult)
                nc.vector.tensor_tensor(t0[:], mg[:], av[:], Op.mult)
                nc.vector.tensor_tensor(bv[:], be_sb[:], t0[:], Op.subtract)
                return av, bv

            # ================= stage 1 =================
            st6_1 = sbuf.tile([P, NB, 2 * BL * 6], F32)
            conv(w1_sb, y_img, yv, st6_1)
            a1v, b1v = stats_to_ab(st6_1, s1_sb, g1_sb, be1_sb, "c1")

            for b in range(BL):
                for m in range(NB):
                    u = y_img[b][:, m, :]
                    nc.vector.tensor_scalar(u, u, a1v[:, m:m + 1],
                                            b1v[:, m:m + 1], Op.mult, Op.add)
                    nc.vector.tensor_tensor(u, u, x_img[b][:, m, :], Op.add)
                    nc.scalar.activation(u, u, F.Prelu, bias=0.0, scale=1.0,
                                         alpha=a1_sb[:, 0:1])
                    # binarize for conv2 (interior only; borders stay zero)
                    nc.scalar.sign(xbv[b][:, m, 1:29, 1:29], yv[b][:, m])

            # ================= stage 2 =================
            st6_2 = sbuf.tile([P, NB, 2 * BL * 6], F32)
            conv(w2_sb, x_img, xv, st6_2)  # y2 overwrites x (residual1 consumed)
            a2v, b2v = stats_to_ab(st6_2, s2_sb, g2_sb, be2_sb, "c2")

            for b in range(BL):
                for m in range(NB):
                    u = x_img[b][:, m, :]
                    nc.vector.tensor_scalar(u, u, a2v[:, m:m + 1],
                                            b2v[:, m:m + 1], Op.mult, Op.add)
                    nc.vector.tensor_tensor(u, u, y_img[b][:, m, :], Op.add)
                    nc.scalar.activation(u, u, F.Prelu, bias=0.0, scale=1.0,
                                         alpha=a2_sb[:, 0:1])
                    for hh in range(2):
                        nc.sync.dma_start(
                            o_d[b, m * P:(m + 1) * P, hh * 14:(hh + 1) * 14, :],
                            xv[b][:, m, hh * 14:(hh + 1) * 14, :])

    nc.compile()
    return nc


def _get_nc():
    if "nc" not in _CACHE:
        _CACHE["nc"] = _build()
    return _CACHE["nc"]


def _pack_w(w):
    wb = np.sign(np.asarray(w, np.float32))
    # [co, ci, kh, kw] -> [ci_blk, ci, tap, co_blk, co]
    t = wb.reshape(NB, P, NB, P, 3, 3)
    t = np.transpose(t, (2, 3, 4, 5, 0, 1)).reshape(NB, P, 9, NB, P)
    return np.ascontiguousarray(t).astype(BF16_NP)


def _pack_vec(v):
    return np.ascontiguousarray(np.asarray(v, np.float32).reshape(NB, P, 1))


def kernel(x, conv1_w, conv2_w, bn1_gamma, bn1_beta, bn2_gamma, bn2_beta,
           prelu1_a, prelu2_a):
    x = np.ascontiguousarray(np.asarray(x, np.float32))
    nc = _get_nc()

    w1p = _pack_w(conv1_w)
    w2p = _pack_w(conv2_w)
    s1 = SCALE * np.mean(np.abs(np.asarray(conv1_w, np.float32)),
                         axis=(1, 2, 3), dtype=np.float32)
    s2 = SCALE * np.mean(np.abs(np.asarray(conv2_w, np.float32)),
                         axis=(1, 2, 3), dtype=np.float32)

    shared = {
        "w1": w1p, "w2": w2p,
        "s1": _pack_vec(s1), "g1": _pack_vec(bn1_gamma),
        "be1": _pack_vec(bn1_beta),
        "s2": _pack_vec(s2), "g2": _pack_vec(bn2_gamma),
        "be2": _pack_vec(bn2_beta),
        "a1": np.asarray(prelu1_a, np.float32).reshape(1),
        "a2": np.asarray(prelu2_a, np.float32).reshape(1),
    }
    in_maps = [dict(shared, x=x[c * BL:(c + 1) * BL]) for c in range(N_CORES)]

    res = bass_utils.run_bass_kernel_spmd(nc, in_maps,
                                          core_ids=list(range(N_CORES)))
    out = np.concatenate([res.results[c]["o"] for c in range(N_CORES)], axis=0)
    return out
